# revision 1
# baseline (speedup 1.0000x reference)
"""ConvQRNN Trainium2 kernel.

Strategy (8 NeuronCores, spatial H-sharding, 8 rows/core):
  - Conv3d(k=(2,3,3), CIN=3 -> 256) lowered to matmul: host builds a fp16
    im2col with K=56 rows (54 taps + ones row carrying the conv bias + one
    zero pad row).  Per timestep each gate's [64ch x 2048pix] pre-activation
    is computed as two col-tiled matmuls (duplicated weight halves) so the
    PSUM layout directly matches the scan layout.
  - Scan layout "B": every scan tensor is [128, 1024] fp16 with
    partition = (b//2)*64 + ch, free = (b%2)*512 + h*64 + w.
  - QRNN cell runs fp16 on DVE/ACT; o-gate/sigmoid/tanh tail is batched over
    8 timesteps; H is written to DRAM as fp16 and upcast on the host.
"""

import os

import numpy as np

B, CIN, T, H, W = 4, 3, 32, 64, 64
COUT = 64
NC = 8
HS = H // NC
K = 56  # 54 conv taps + ones(bias) row + zero pad row
PIX = B * HS * W          # 2048 pixels per core per timestep
F = PIX // 2              # 1024 free elements per tile
KAPPA = 8                 # o-gate tail batch (timesteps)

f16 = np.float16

_CACHE = {}

LAST_RESULTS = {}


def _host_prep(X, Wconv, bconv, W_ci, W_cf, W_co):
    X = np.ascontiguousarray(np.asarray(X, np.float32))
    Wconv = np.asarray(Wconv, np.float32)
    bconv = np.asarray(bconv, np.float32)
    Xp = np.pad(X, ((0, 0), (0, 0), (1, 0), (1, 1), (1, 1)))  # (B,CIN,T+1,H+2,W+2)

    im2col = np.zeros((NC, K, T, PIX), f16)
    for c in range(NC):
        for cin in range(CIN):
            for dt in range(2):
                for dh in range(3):
                    for dw in range(3):
                        k = ((cin * 2 + dt) * 3 + dh) * 3 + dw
                        blk = Xp[:, cin, dt:dt + T,
                                 8 * c + dh:8 * c + dh + HS, dw:dw + W]
                        blk = blk.reshape(2, 2, T, HS, W).transpose(2, 0, 1, 3, 4)
                        im2col[c, k] = blk.reshape(T, PIX).astype(f16)
        im2col[c, 54] = 1.0

    lhsT = np.zeros((4, K, 128), f16)
    Wr = Wconv.reshape(4, COUT, CIN, 2, 3, 3)
    for g in range(4):
        wk = Wr[g].transpose(1, 2, 3, 4, 0).reshape(54, COUT).astype(f16)
        lhsT[g, :54, :64] = wk
        lhsT[g, :54, 64:] = wk
        lhsT[g, 54, :64] = bconv[g * 64:(g + 1) * 64].astype(f16)
        lhsT[g, 54, 64:] = bconv[g * 64:(g + 1) * 64].astype(f16)

    peep = np.zeros((NC, 3, 128, F), f16)
    for i, Wc in enumerate((W_ci, W_cf, W_co)):
        Wc = np.asarray(Wc, np.float32)
        for c in range(NC):
            sl = Wc[:, 8 * c:8 * c + HS, :].reshape(64, HS * W).astype(f16)
            tile = np.empty((128, F), f16)
            for half in range(2):
                for b1 in range(2):
                    tile[64 * half:64 * half + 64, 512 * b1:512 * b1 + 512] = sl
            peep[c, i] = tile
    return im2col, lhsT, peep


def _build_nc(loop_reps=1):
    import concourse.bacc as bacc
    import concourse.mybir as mybir
    from contextlib import nullcontext
    from concourse.tile import TileContext

    fp16 = mybir.dt.float16
    fp32 = mybir.dt.float32
    AF = mybir.ActivationFunctionType

    nc = bacc.Bacc(None, target_bir_lowering=False)

    im2col_d = nc.dram_tensor("im2col", [K, T, PIX], fp16, kind="ExternalInput")
    lhsT_d = nc.dram_tensor("lhsT", [4, K, 128], fp16, kind="ExternalInput")
    peep_d = nc.dram_tensor("peep", [3, 128, F], fp16, kind="ExternalInput")
    out_d = nc.dram_tensor("out", [T, 128, F], fp16, kind="ExternalOutput")

    with TileContext(nc) as tc:
        with (
            tc.tile_pool(name="const", bufs=1) as constp,
            tc.tile_pool(name="state", bufs=1) as statep,
            tc.tile_pool(name="rhs", bufs=3) as rhsp,
            tc.tile_pool(name="work", bufs=2) as workp,
            tc.tile_pool(name="tail", bufs=1) as tailp,
            tc.tile_pool(name="psum", bufs=1, space="PSUM") as psump,
        ):
            wci = constp.tile([128, F], fp16)
            wcf = constp.tile([128, F], fp16)
            wco = constp.tile([128, F], fp16)
            nc.sync.dma_start(out=wci[:], in_=peep_d[0])
            nc.sync.dma_start(out=wcf[:], in_=peep_d[1])
            nc.sync.dma_start(out=wco[:], in_=peep_d[2])
            lhsT_sb = constp.tile([K, 4 * 128], fp16)
            nc.sync.dma_start(
                out=lhsT_sb[:].rearrange("k (g m) -> k g m", g=4),
                in_=lhsT_d[:].rearrange("g k m -> k g m"),
            )

            # C ring: slot s holds C_{8k+s-1}; slot 0 seeded with zeros /
            # previous window's last state.
            c_hist = statep.tile([128, (KAPPA + 1) * F], fp16)
            a_o_hist = statep.tile([128, KAPPA * F], fp16)
            nc.vector.memset(c_hist[:, 0:F], 0.0)

            e_if = psump.tile([128, 2 * F], fp32)
            e_g = psump.tile([128, F], fp32)
            e_o = psump.tile([128, F], fp32)

            loop_cm = tc.For_i(0, loop_reps) if loop_reps > 1 else nullcontext()
            with loop_cm:
                for t in range(T):
                        j = t % KAPPA
                        c_prev = c_hist[:, j * F:(j + 1) * F]
                        c_next = c_hist[:, (j + 1) * F:(j + 2) * F]

                        rhs = rhsp.tile([K, PIX], fp16)
                        nc.sync.dma_start(out=rhs[:], in_=im2col_d[:, t, :])

                        # gates: i -> e_if[:, 0:F], f -> e_if[:, F:2F], g, o
                        for g, (ptile, foff) in enumerate(
                            ((e_if, 0), (e_if, F), (e_g, 0), (e_o, 0))
                        ):
                            for hf in range(2):
                                lw = lhsT_sb[:, g * 128 + 64 * hf:g * 128 + 64 * hf + 64]
                                for q in range(2):
                                    nc.tensor.matmul(
                                        ptile[64 * hf:64 * hf + 64,
                                              foff + 512 * q:foff + 512 * q + 512],
                                        lw,
                                        rhs[:, 1024 * hf + 512 * q:1024 * hf + 512 * q + 512],
                                        start=True,
                                        stop=True,
                                        tile_position=(0, 64 * hf),
                                    )

                        v_if = workp.tile([128, 2 * F], fp16)
                        nc.vector.tensor_mul(out=v_if[:, 0:F], in0=wci[:], in1=c_prev)
                        nc.vector.tensor_mul(out=v_if[:, F:2 * F], in0=wcf[:], in1=c_prev)
                        a_if = workp.tile([128, 2 * F], fp16)
                        nc.vector.tensor_add(out=a_if[:], in0=e_if[:], in1=v_if[:])
                        s_if = workp.tile([128, 2 * F], fp16)
                        nc.scalar.activation(s_if[:], a_if[:], AF.Sigmoid)
                        t_g = workp.tile([128, F], fp16)
                        nc.scalar.activation(t_g[:], e_g[:], AF.Tanh)
                        p1 = workp.tile([128, F], fp16)
                        nc.vector.tensor_mul(out=p1[:], in0=s_if[:, 0:F], in1=t_g[:])
                        p2 = workp.tile([128, F], fp16)
                        nc.vector.tensor_mul(out=p2[:], in0=s_if[:, F:2 * F], in1=c_prev)
                        nc.vector.tensor_add(out=c_next, in0=p1[:], in1=p2[:])

                        v_o = workp.tile([128, F], fp16)
                        nc.vector.tensor_mul(out=v_o[:], in0=wco[:], in1=c_next)
                        nc.vector.tensor_add(
                            out=a_o_hist[:, j * F:(j + 1) * F], in0=e_o[:], in1=v_o[:]
                        )

                        if j == KAPPA - 1:
                            s_o = tailp.tile([128, KAPPA * F], fp16)
                            nc.scalar.activation(s_o[:], a_o_hist[:], AF.Sigmoid)
                            t_c = tailp.tile([128, KAPPA * F], fp16)
                            nc.scalar.activation(t_c[:], c_hist[:, F:(KAPPA + 1) * F], AF.Tanh)
                            h8 = tailp.tile([128, KAPPA * F], fp16)
                            nc.vector.tensor_mul(out=h8[:], in0=s_o[:], in1=t_c[:])
                            k0 = t - KAPPA + 1
                            nc.sync.dma_start(
                                out=out_d[k0:k0 + KAPPA].rearrange("t p f -> p t f"),
                                in_=h8[:].rearrange("p (t f) -> p t f", t=KAPPA),
                            )
                            if t != T - 1:
                                # carry last state of the window into ring slot 0
                                nc.vector.tensor_copy(
                                    out=c_hist[:, 0:F],
                                    in_=c_hist[:, KAPPA * F:(KAPPA + 1) * F],
                                )

    nc.compile()
    return nc


def _get_nc():
    if "nc" not in _CACHE:
        _CACHE["nc"] = _build_nc()
    return _CACHE["nc"]


def kernel(X, Wconv, bconv, W_ci, W_cf, W_co):
    from concourse.bass_utils import run_bass_kernel_spmd

    im2col, lhsT, peep = _host_prep(X, Wconv, bconv, W_ci, W_cf, W_co)
    nc = _get_nc()
    in_maps = [
        {"im2col": im2col[c], "lhsT": lhsT, "peep": peep[c]} for c in range(NC)
    ]
    trace = bool(os.environ.get("QRNN_TRACE"))
    res = run_bass_kernel_spmd(
        nc, in_maps, core_ids=list(range(NC)), trace=trace
    )
    LAST_RESULTS["exec_time_ns"] = getattr(res, "exec_time_ns", None)

    O = np.empty((B, COUT, T, H, W), np.float32)
    for c in range(NC):
        o = np.asarray(res.results[c]["out"], f16).astype(np.float32)
        o = o.reshape(T, 2, 64, 2, HS, W).transpose(1, 3, 2, 0, 4, 5)
        O[:, :, :, 8 * c:8 * c + HS, :] = o.reshape(B, COUT, T, HS, W)
    return O



# revision 12
# speedup vs baseline: 1.0952x; 1.0952x over previous
"""ConvQRNN Trainium2 kernel.

Strategy (8 NeuronCores, spatial H-sharding, 8 rows/core):
  - Conv3d(k=(2,3,3), CIN=3 -> 256) lowered to matmul: host builds a fp16
    im2col with K=56 rows (54 taps + ones row carrying the conv bias + one
    zero pad row).  Per timestep each gate's [64ch x 2048pix] pre-activation
    is computed as two col-tiled matmuls (duplicated weight halves) so the
    PSUM layout directly matches the scan layout.
  - Scan layout: [128, 1024] fp16 tiles with partition = (b//2)*64 + ch,
    free = (b%2)*512 + h*64 + w.  The two column halves (q = b%2) are
    independent scans and run as two interleaved dependency chains so the
    DVE / ACT / Pool engines overlap instead of ping-ponging.
  - Per half-chain and step: one fused peephole mul (stride-0 broadcast of
    C gives [Wcf|Wci] * [C;C] in a single 1024-col op), one PSUM-draining
    add, one 1024-col sigmoid, one fused [s_f|s_i]*[C|tanh(g)] mul (tanh(g)
    is written into the ring slot that C_next then overwrites), one 512-col
    add.  The o-gate (Wco*C, PSUM drain) runs on the otherwise-idle GpSimd
    engine; tanh(C) runs per step on ACT.
  - The output tail (sigmoid(a_o), H = s_o * tanh(C)) for window w is
    spread one 512-col chunk per step across window w+1 (ping-pong
    buffers), so no engine stalls on a batched tail.
"""

import os

import numpy as np

B, CIN, T, H, W = 4, 3, 32, 64, 64
COUT = 64
NC = 8
HS = H // NC
K = 56  # 54 conv taps + ones(bias) row + zero pad row
PIX = B * HS * W          # 2048 pixels per core per timestep
F = PIX // 2              # 1024 free elements per [128, F] tile
FH = F // 2               # 512 cols per half-chain
KAPPA = 8                 # output window (timesteps)
NW = T // KAPPA

f16 = np.float16

_CACHE = {}

LAST_RESULTS = {}


def _host_prep(X, Wconv, bconv, W_ci, W_cf, W_co):
    X = np.ascontiguousarray(np.asarray(X, np.float32))
    Wconv = np.asarray(Wconv, np.float32)
    bconv = np.asarray(bconv, np.float32)
    Xp = np.pad(X, ((0, 0), (0, 0), (1, 0), (1, 1), (1, 1)))  # (B,CIN,T+1,H+2,W+2)

    im2col = np.zeros((NC, K, T, PIX), f16)
    for c in range(NC):
        for cin in range(CIN):
            for dt in range(2):
                for dh in range(3):
                    for dw in range(3):
                        k = ((cin * 2 + dt) * 3 + dh) * 3 + dw
                        blk = Xp[:, cin, dt:dt + T,
                                 8 * c + dh:8 * c + dh + HS, dw:dw + W]
                        blk = blk.reshape(2, 2, T, HS, W).transpose(2, 0, 1, 3, 4)
                        im2col[c, k] = blk.reshape(T, PIX).astype(f16)
        im2col[c, 54] = 1.0

    # gate order in the kernel is (f, i, g, o)
    gate_order = (1, 0, 2, 3)
    lhsT = np.zeros((4, K, 128), f16)
    Wr = Wconv.reshape(4, COUT, CIN, 2, 3, 3)
    for gi, g in enumerate(gate_order):
        wk = Wr[g].transpose(1, 2, 3, 4, 0).reshape(54, COUT).astype(f16)
        lhsT[gi, :54, :64] = wk
        lhsT[gi, :54, 64:] = wk
        lhsT[gi, 54, :64] = bconv[g * 64:(g + 1) * 64].astype(f16)
        lhsT[gi, 54, 64:] = bconv[g * 64:(g + 1) * 64].astype(f16)

    # peep[c]: [128, 3*FH] = [Wcf | Wci | Wco], rows duplicated over the two
    # row-halves (both hold the same 64 channels).
    peep = np.zeros((NC, 128, 3 * FH), f16)
    for c in range(NC):
        for i, Wc in enumerate((W_cf, W_ci, W_co)):
            sl = np.asarray(Wc, np.float32)[:, 8 * c:8 * c + HS, :]
            sl = sl.reshape(64, FH).astype(f16)
            peep[c, :64, i * FH:(i + 1) * FH] = sl
            peep[c, 64:, i * FH:(i + 1) * FH] = sl
    return im2col, lhsT, peep


def _build_nc():
    import concourse.bacc as bacc
    import concourse.mybir as mybir
    from concourse.tile import TileContext

    fp16 = mybir.dt.float16
    fp32 = mybir.dt.float32
    AF = mybir.ActivationFunctionType

    nc = bacc.Bacc(None, target_bir_lowering=False)

    im2col_d = nc.dram_tensor("im2col", [K, T, PIX], fp16, kind="ExternalInput")
    lhsT_d = nc.dram_tensor("lhsT", [4, K, 128], fp16, kind="ExternalInput")
    peep_d = nc.dram_tensor("peep", [128, 3 * FH], fp16, kind="ExternalInput")
    out_d = nc.dram_tensor("out", [T, 128, F], fp16, kind="ExternalOutput")

    def bcast2(ap_512):
        # [128, FH] -> [128, 2, FH] with stride-0 middle dim
        return ap_512.rearrange("p (o f) -> p o f", o=1).broadcast_to([128, 2, FH])

    with TileContext(nc) as tc:
        with (
            tc.tile_pool(name="const", bufs=1) as constp,
            tc.tile_pool(name="state", bufs=1) as statep,
            tc.tile_pool(name="rhs", bufs=3) as rhsp,
            tc.tile_pool(name="work", bufs=3) as workp,
            tc.tile_pool(name="pwork", bufs=3) as pworkp,
            tc.tile_pool(name="psum", bufs=1, space="PSUM") as psump,
        ):
            wcif = constp.tile([128, F], fp16)      # [Wcf | Wci]
            wco = constp.tile([128, FH], fp16)
            nc.sync.dma_start(out=wcif[:], in_=peep_d[:, 0:F])
            nc.sync.dma_start(out=wco[:], in_=peep_d[:, F:F + FH])
            lhsT_sb = constp.tile([K, 4 * 128], fp16)
            nc.sync.dma_start(
                out=lhsT_sb[:].rearrange("k (g m) -> k g m", g=4),
                in_=lhsT_d[:].rearrange("g k m -> k g m"),
            )
            zero5 = constp.tile([128, FH], fp16)
            nc.vector.memset(zero5[:], 0.0)

            # per (q, parity) state: ring of C (slots 1..8), a_o history,
            # tanh(C), s_o*tanh(C) output chunks
            ring = [[statep.tile([128, KAPPA * FH], fp16, name=f"ring{q}{p}")
                     for p in range(2)] for q in range(2)]
            aoh = [[statep.tile([128, KAPPA * FH], fp16, name=f"aoh{q}{p}")
                    for p in range(2)] for q in range(2)]
            tch = [[statep.tile([128, KAPPA * FH], fp16, name=f"tch{q}{p}")
                    for p in range(2)] for q in range(2)]
            h8 = [[statep.tile([128, KAPPA * FH], fp16, name=f"h8{q}{p}")
                   for p in range(2)] for q in range(2)]

            e_if = [psump.tile([128, F], fp32, name=f"eif{q}") for q in range(2)]
            e_g = [psump.tile([128, FH], fp32, name=f"eg{q}") for q in range(2)]
            e_o = [psump.tile([128, FH], fp32, name=f"eo{q}") for q in range(2)]

            for t in range(T):
                j = t % KAPPA
                w = t // KAPPA
                par = w % 2
                ppar = 1 - par

                rhs = rhsp.tile([K, PIX], fp16)
                nc.sync.dma_start(out=rhs[:], in_=im2col_d[:, t, :])

                # o-gate add of the PREVIOUS step (PSUM drain must be DVE;
                # deferred one step so Pool's v_o latency never stalls DVE).
                # Emitted BEFORE this step's matmuls so it reads the old e_o.
                if t > 0:
                    pj, pw = (t - 1) % KAPPA, (t - 1) // KAPPA
                    for q in range(2):
                        nc.vector.tensor_add(
                            out=aoh[q][pw % 2][:, pj * FH:(pj + 1) * FH],
                            in0=e_o[q][:], in1=vo_prev[q][:],
                        )

                # conv matmuls: gates (f, i, g, o); o last (its PSUM bank
                # drains latest in the previous step)
                for gi in range(4):
                    for hf in range(2):
                        lw = lhsT_sb[:, gi * 128 + 64 * hf:gi * 128 + 64 * hf + 64]
                        for q in range(2):
                            if gi == 0:
                                tgt = e_if[q][64 * hf:64 * hf + 64, 0:FH]
                            elif gi == 1:
                                tgt = e_if[q][64 * hf:64 * hf + 64, FH:F]
                            elif gi == 2:
                                tgt = e_g[q][64 * hf:64 * hf + 64, :]
                            else:
                                tgt = e_o[q][64 * hf:64 * hf + 64, :]
                            b = 2 * hf + q
                            nc.tensor.matmul(
                                tgt,
                                lw,
                                rhs[:, b * FH:(b + 1) * FH],
                                start=True,
                                stop=True,
                                tile_position=(0, 64 * hf),
                            )

                vs = []
                ss = []
                for q in range(2):
                    if t == 0:
                        c_prev = zero5[:]
                    elif j == 0:
                        c_prev = ring[q][ppar][:, 7 * FH:8 * FH]
                    else:
                        c_prev = ring[q][par][:, (j - 1) * FH:j * FH]

                    # v = [Wcf|Wci] * [C;C]   (one 1024-col 2x-mode op)
                    v = workp.tile([128, F], fp16)
                    nc.vector.tensor_mul(
                        out=v[:].rearrange("p (o f) -> p o f", o=2),
                        in0=wcif[:].rearrange("p (o f) -> p o f", o=2),
                        in1=bcast2(c_prev),
                    )
                    # a = e_if + v   (PSUM drain, in-place over v)
                    nc.vector.tensor_add(out=v[:], in0=e_if[q][:], in1=v[:])
                    s = workp.tile([128, F], fp16)
                    nc.scalar.activation(s[:], v[:], AF.Sigmoid)
                    # tanh(g) -> ring slot j+1 (overwritten by C_next below)
                    nc.scalar.activation(
                        ring[q][par][:, j * FH:(j + 1) * FH], e_g[q][:], AF.Tanh
                    )
                    vs.append(v)
                    ss.append(s)

                vo_prev = []
                for q in range(2):
                    v, s = vs[q], ss[q]
                    # pp = [s_f|s_i] * [C_prev | tanh(g)] (in-place over v)
                    if j > 0:
                        nc.vector.tensor_mul(
                            out=v[:],
                            in0=s[:],
                            in1=ring[q][par][:, (j - 1) * FH:(j + 1) * FH],
                        )
                    else:
                        cp = zero5[:] if t == 0 else ring[q][ppar][:, 7 * FH:8 * FH]
                        nc.vector.tensor_mul(out=v[:, 0:FH], in0=s[:, 0:FH], in1=cp)
                        nc.vector.tensor_mul(
                            out=v[:, FH:F], in0=s[:, FH:F],
                            in1=ring[q][par][:, 0:FH],
                        )
                    # C_next = s_f*C + s_i*tanh(g) -> ring slot j+1 (GpSimd)
                    cn = ring[q][par][:, j * FH:(j + 1) * FH]
                    nc.gpsimd.tensor_add(out=cn, in0=v[:, 0:FH], in1=v[:, FH:F])
                    # tanh(C_next) for the output tail
                    nc.scalar.activation(
                        tch[q][par][:, j * FH:(j + 1) * FH], cn, AF.Tanh
                    )
                    # v_o = Wco*C_next on GpSimd; the PSUM add happens next step
                    vo = pworkp.tile([128, FH], fp16)
                    nc.gpsimd.tensor_mul(out=vo[:], in0=wco[:], in1=cn)
                    vo_prev.append(vo)

                # previous window's output tail, one chunk per step
                if w > 0:
                    for q in range(2):
                        sl = slice(j * FH, (j + 1) * FH)
                        so_c = h8[q][ppar][:, sl]
                        nc.scalar.activation(so_c, aoh[q][ppar][:, sl], AF.Sigmoid)
                        eng = nc.vector if q == 0 else nc.gpsimd
                        eng.tensor_mul(out=so_c, in0=so_c, in1=tch[q][ppar][:, sl])
                    if j == KAPPA - 1:
                        k0 = (w - 1) * KAPPA
                        for q in range(2):
                            nc.sync.dma_start(
                                out=out_d[k0:k0 + KAPPA, :, q * FH:(q + 1) * FH]
                                .rearrange("t p f -> p t f"),
                                in_=h8[q][ppar][:]
                                .rearrange("p (t f) -> p t f", t=KAPPA),
                            )

            # last step's deferred o-gate add, then the final window's tail
            par = (NW - 1) % 2
            k0 = (NW - 1) * KAPPA
            for q in range(2):
                nc.vector.tensor_add(
                    out=aoh[q][par][:, (KAPPA - 1) * FH:KAPPA * FH],
                    in0=e_o[q][:], in1=vo_prev[q][:],
                )
            for q in range(2):
                nc.scalar.activation(h8[q][par][:], aoh[q][par][:], AF.Sigmoid)
                nc.vector.tensor_mul(
                    out=h8[q][par][:], in0=h8[q][par][:], in1=tch[q][par][:]
                )
                nc.sync.dma_start(
                    out=out_d[k0:k0 + KAPPA, :, q * FH:(q + 1) * FH]
                    .rearrange("t p f -> p t f"),
                    in_=h8[q][par][:].rearrange("p (t f) -> p t f", t=KAPPA),
                )

    nc.compile()
    return nc


def _get_nc():
    if "nc" not in _CACHE:
        _CACHE["nc"] = _build_nc()
    return _CACHE["nc"]


def kernel(X, Wconv, bconv, W_ci, W_cf, W_co):
    from concourse.bass_utils import run_bass_kernel_spmd

    im2col, lhsT, peep = _host_prep(X, Wconv, bconv, W_ci, W_cf, W_co)
    nc = _get_nc()
    in_maps = [
        {"im2col": im2col[c], "lhsT": lhsT, "peep": peep[c]} for c in range(NC)
    ]
    trace = bool(os.environ.get("QRNN_TRACE"))
    res = run_bass_kernel_spmd(
        nc, in_maps, core_ids=list(range(NC)), trace=trace
    )
    LAST_RESULTS["exec_time_ns"] = getattr(res, "exec_time_ns", None)

    O = np.empty((B, COUT, T, H, W), np.float32)
    for c in range(NC):
        o = np.asarray(res.results[c]["out"], f16).astype(np.float32)
        o = o.reshape(T, 2, 64, 2, HS, W).transpose(1, 3, 2, 0, 4, 5)
        O[:, :, :, 8 * c:8 * c + HS, :] = o.reshape(B, COUT, T, HS, W)
    return O


# revision 15
# speedup vs baseline: 1.1391x; 1.0400x over previous
"""ConvQRNN Trainium2 kernel.

Strategy (8 NeuronCores, spatial H-sharding, 8 rows/core):
  - Conv3d(k=(2,3,3), CIN=3 -> 256) lowered to matmul: host builds a fp16
    im2col with K=56 rows (54 taps + ones row carrying the conv bias + one
    zero pad row).
  - Scan layout: [128, 1024] fp16 per step with partition = (b//2)*64 + ch,
    free = (b%2)*512 + h*64 + w.  The two column halves (q = b%2) are
    independent scans interleaved as two dependency chains.
  - C state lives at a FIXED address in ctb = [C0|tg0|C1|tg1]; tanh(g) is
    written next to C so the fused [s_f|s_i]*[C|tanh(g)] multiply reads one
    contiguous operand.  All fp16 DVE ops keep src0/src1/dst congruent
    mod 4KB, which HW requires for the 2x DVE mode.
  - o-gate: DVE writes Wco*C into the o PSUM bank, then the o-gate conv
    matmul ACCUMULATES on top (start=False), so sigmoid reads the finished
    pre-activation straight from PSUM.  No separate o-gate add.
  - Off-chain ops (tanh g, tanh C, sigmoid o, H-mul) are merged across the
    two halves into single 1024-col ops and deferred one step so they never
    stall the recurrence chain.
"""

import os

import numpy as np

B, CIN, T, H, W = 4, 3, 32, 64, 64
COUT = 64
NC = 8
HS = H // NC
K = 56
PIX = B * HS * W          # 2048
F = PIX // 2              # 1024
FH = F // 2               # 512
KAPPA = 8
NW = T // KAPPA

f16 = np.float16

_CACHE = {}
LAST_RESULTS = {}


def _host_prep(X, Wconv, bconv, W_ci, W_cf, W_co):
    X = np.ascontiguousarray(np.asarray(X, np.float32))
    Wconv = np.asarray(Wconv, np.float32)
    bconv = np.asarray(bconv, np.float32)
    Xp = np.pad(X, ((0, 0), (0, 0), (1, 0), (1, 1), (1, 1)))

    im2col = np.zeros((NC, K, T, PIX), f16)
    for c in range(NC):
        for cin in range(CIN):
            for dt in range(2):
                for dh in range(3):
                    for dw in range(3):
                        k = ((cin * 2 + dt) * 3 + dh) * 3 + dw
                        blk = Xp[:, cin, dt:dt + T,
                                 8 * c + dh:8 * c + dh + HS, dw:dw + W]
                        blk = blk.reshape(2, 2, T, HS, W).transpose(2, 0, 1, 3, 4)
                        im2col[c, k] = blk.reshape(T, PIX).astype(f16)
        im2col[c, 54] = 1.0

    # kernel gate order: (f, i, g, o)
    gate_order = (1, 0, 2, 3)
    lhsT = np.zeros((4, K, 128), f16)
    Wr = Wconv.reshape(4, COUT, CIN, 2, 3, 3)
    for gi, g in enumerate(gate_order):
        wk = Wr[g].transpose(1, 2, 3, 4, 0).reshape(54, COUT).astype(f16)
        lhsT[gi, :54, :64] = wk
        lhsT[gi, :54, 64:] = wk
        lhsT[gi, 54, :64] = bconv[g * 64:(g + 1) * 64].astype(f16)
        lhsT[gi, 54, 64:] = bconv[g * 64:(g + 1) * 64].astype(f16)

    # peep[c]: [128, 2048] = [Wcf | Wci | Wco | Wco], rows duplicated over
    # the two row-halves (both hold the same 64 channels).
    peep = np.zeros((NC, 128, 4 * FH), f16)
    for c in range(NC):
        for i, Wc in enumerate((W_cf, W_ci, W_co, W_co)):
            sl = np.asarray(Wc, np.float32)[:, 8 * c:8 * c + HS, :]
            sl = sl.reshape(64, FH).astype(f16)
            peep[c, :64, i * FH:(i + 1) * FH] = sl
            peep[c, 64:, i * FH:(i + 1) * FH] = sl
    return im2col, lhsT, peep


def _build_nc():
    import concourse.bacc as bacc
    import concourse.mybir as mybir
    from concourse.tile import TileContext

    fp16 = mybir.dt.float16
    fp32 = mybir.dt.float32
    AF = mybir.ActivationFunctionType

    nc = bacc.Bacc(None, target_bir_lowering=False)

    im2col_d = nc.dram_tensor("im2col", [K, T, PIX], fp16, kind="ExternalInput")
    lhsT_d = nc.dram_tensor("lhsT", [4, K, 128], fp16, kind="ExternalInput")
    peep_d = nc.dram_tensor("peep", [128, 4 * FH], fp16, kind="ExternalInput")
    out_d = nc.dram_tensor("out", [T, 128, F], fp16, kind="ExternalOutput")

    with TileContext(nc) as tc:
        with (
            tc.tile_pool(name="const", bufs=1) as constp,
            tc.tile_pool(name="al", bufs=1) as alp,
            tc.tile_pool(name="rhs", bufs=3) as rhsp,
            tc.tile_pool(name="psum", bufs=1, space="PSUM") as psump,
        ):
            wcif = constp.tile([128, F], fp16)       # [Wcf | Wci]
            wcoD = constp.tile([128, F], fp16)       # [Wco | Wco]
            nc.sync.dma_start(out=wcif[:], in_=peep_d[:, 0:F])
            nc.sync.dma_start(out=wcoD[:], in_=peep_d[:, F:2 * F])
            lhsT_sb = constp.tile([K, 4 * 128], fp16)
            nc.sync.dma_start(
                out=lhsT_sb[:].rearrange("k (g m) -> k g m", g=4),
                in_=lhsT_d[:].rearrange("g k m -> k g m"),
            )

            # aligned arena: every tile a 4KB multiple so all bases (and
            # equal-offset slices) stay congruent mod 4KB -> DVE 2x mode
            vv = alp.tile([128, 2 * F], fp16)        # [vf0|vi0|vf1|vi1]
            ss = alp.tile([128, 2 * F], fp16)        # sigmoid outputs
            ctb = alp.tile([128, 2 * F], fp16)       # [C0|tg0|C1|tg1]
            soh = alp.tile([128, KAPPA * F], fp16)   # sigmoid(a_o) slots
            tch = alp.tile([128, KAPPA * F], fp16)   # tanh(C) slots
            h8 = [alp.tile([128, KAPPA * F], fp16, name=f"h8{p}")
                  for p in range(2)]

            e_if = psump.tile([128, 2 * F], fp32)    # [f0|i0|f1|i1]
            e_g = psump.tile([128, F], fp32)         # [g0|g1]
            e_o = psump.tile([128, F], fp32)         # [o0|o1]

            nc.vector.memset(ctb[:, 0:FH], 0.0)
            nc.vector.memset(ctb[:, F:F + FH], 0.0)

            def cslice(q):
                return ctb[:, q * F:q * F + FH]

            def c2seg():
                # [C0 | C1] as a 2-segment strided AP
                return ctb[:].rearrange("p (s f) -> p s f", s=2)[:, :, 0:FH]

            def tg2seg():
                # [tg0 | tg1]
                return ctb[:].rearrange("p (s f) -> p s f", s=2)[:, :, FH:F]

            rhs_t = {}
            for t in range(T):
                j = t % KAPPA
                w = t // KAPPA
                par = w % 2

                rhs = rhsp.tile([K, PIX], fp16)
                nc.sync.dma_start(out=rhs[:], in_=im2col_d[:, t, :])
                rhs_t[t] = rhs

                # ---- previous step's o-gate + output tail ----
                if t > 0:
                    pj = (t - 1) % KAPPA
                    ppar = ((t - 1) // KAPPA) % 2
                    # Wco*C(t-1) into the o PSUM banks (before o-matmuls)
                    nc.vector.tensor_mul(
                        out=e_o[:].rearrange("p (s f) -> p s f", s=2),
                        in0=wcoD[:].rearrange("p (s f) -> p s f", s=2),
                        in1=c2seg(),
                    )
                    # o-gate conv accumulates on top
                    for hf in range(2):
                        lw = lhsT_sb[:, 3 * 128 + 64 * hf:3 * 128 + 64 * hf + 64]
                        for q in range(2):
                            b = 2 * hf + q
                            nc.tensor.matmul(
                                e_o[64 * hf:64 * hf + 64, q * FH:(q + 1) * FH],
                                lw,
                                rhs_t[t - 1][:, b * FH:(b + 1) * FH],
                                start=False,
                                stop=True,
                                tile_position=(0, 64 * hf),
                            )
                    del rhs_t[t - 1]
                    # tanh(C(t-1)) -> tch slot (before Cn(t) overwrites C)
                    nc.scalar.activation(
                        tch[:, pj * F:(pj + 1) * F],
                        c2seg(), AF.Tanh,
                    )
                    # sigmoid(a_o) straight from PSUM
                    nc.scalar.activation(
                        soh[:, pj * F:(pj + 1) * F], e_o[:], AF.Sigmoid
                    )
                    # H chunk on GpSimd
                    nc.gpsimd.tensor_mul(
                        out=h8[ppar][:, pj * F:(pj + 1) * F],
                        in0=soh[:, pj * F:(pj + 1) * F],
                        in1=tch[:, pj * F:(pj + 1) * F],
                    )
                    if pj == KAPPA - 1:
                        k0 = ((t - 1) // KAPPA) * KAPPA
                        for q in range(2):
                            nc.sync.dma_start(
                                out=out_d[k0:k0 + KAPPA, :, q * FH:(q + 1) * FH]
                                .rearrange("t p f -> p t f"),
                                in_=h8[ppar][:]
                                .rearrange("p (t s f) -> p t s f",
                                           t=KAPPA, s=2)[:, :, q, :],
                            )

                # ---- this step's f/i/g conv matmuls ----
                for gi in range(3):
                    for hf in range(2):
                        lw = lhsT_sb[:, gi * 128 + 64 * hf:gi * 128 + 64 * hf + 64]
                        for q in range(2):
                            if gi < 2:
                                tgt = e_if[64 * hf:64 * hf + 64,
                                           q * F + gi * FH:q * F + (gi + 1) * FH]
                            else:
                                tgt = e_g[64 * hf:64 * hf + 64,
                                          q * FH:(q + 1) * FH]
                            b = 2 * hf + q
                            nc.tensor.matmul(
                                tgt, lw, rhs[:, b * FH:(b + 1) * FH],
                                start=True, stop=True,
                                tile_position=(0, 64 * hf),
                            )

                # tanh(g) -> tg slots of ctb (2-segment dst)
                nc.scalar.activation(tg2seg(), e_g[:], AF.Tanh)

                # ---- recurrence chains (per half q) ----
                for q in range(2):
                    # v = [Wcf|Wci] * [C;C]  (broadcast src1 -> fast)
                    nc.vector.tensor_mul(
                        out=vv[:, q * F:(q + 1) * F]
                        .rearrange("p (o f) -> p o f", o=2),
                        in0=wcif[:].rearrange("p (o f) -> p o f", o=2),
                        in1=cslice(q).rearrange("p (o f) -> p o f", o=1)
                        .broadcast_to([128, 2, FH]),
                    )
                    # a = e_if + v  (PSUM drain, in-place)
                    nc.vector.tensor_add(
                        out=vv[:, q * F:(q + 1) * F],
                        in0=e_if[:, q * F:(q + 1) * F],
                        in1=vv[:, q * F:(q + 1) * F],
                    )
                    nc.scalar.activation(
                        ss[:, q * F:(q + 1) * F],
                        vv[:, q * F:(q + 1) * F], AF.Sigmoid,
                    )
                for q in range(2):
                    # pp = [s_f|s_i] * [C|tg]  (all operands congruent mod 4KB)
                    nc.vector.tensor_mul(
                        out=vv[:, q * F:(q + 1) * F],
                        in0=ss[:, q * F:(q + 1) * F],
                        in1=ctb[:, q * F:(q + 1) * F],
                    )
                    # C_next = s_f*C + s_i*tg  (GpSimd, fixed address)
                    nc.gpsimd.tensor_add(
                        out=cslice(q),
                        in0=vv[:, q * F:q * F + FH],
                        in1=vv[:, q * F + FH:(q + 1) * F],
                    )

            # ---- epilogue: last step's o-gate + tail ----
            t = T - 1
            pj = t % KAPPA
            ppar = (t // KAPPA) % 2
            nc.vector.tensor_mul(
                out=e_o[:].rearrange("p (s f) -> p s f", s=2),
                in0=wcoD[:].rearrange("p (s f) -> p s f", s=2),
                in1=c2seg(),
            )
            for hf in range(2):
                lw = lhsT_sb[:, 3 * 128 + 64 * hf:3 * 128 + 64 * hf + 64]
                for q in range(2):
                    b = 2 * hf + q
                    nc.tensor.matmul(
                        e_o[64 * hf:64 * hf + 64, q * FH:(q + 1) * FH],
                        lw,
                        rhs_t[t][:, b * FH:(b + 1) * FH],
                        start=False, stop=True,
                        tile_position=(0, 64 * hf),
                    )
            nc.scalar.activation(tch[:, pj * F:(pj + 1) * F], c2seg(), AF.Tanh)
            nc.scalar.activation(soh[:, pj * F:(pj + 1) * F], e_o[:], AF.Sigmoid)
            nc.gpsimd.tensor_mul(
                out=h8[ppar][:, pj * F:(pj + 1) * F],
                in0=soh[:, pj * F:(pj + 1) * F],
                in1=tch[:, pj * F:(pj + 1) * F],
            )
            k0 = (NW - 1) * KAPPA
            for q in range(2):
                nc.sync.dma_start(
                    out=out_d[k0:k0 + KAPPA, :, q * FH:(q + 1) * FH]
                    .rearrange("t p f -> p t f"),
                    in_=h8[ppar][:]
                    .rearrange("p (t s f) -> p t s f", t=KAPPA, s=2)[:, :, q, :],
                )

    nc.compile()
    return nc


def _get_nc():
    if "nc" not in _CACHE:
        _CACHE["nc"] = _build_nc()
    return _CACHE["nc"]


def kernel(X, Wconv, bconv, W_ci, W_cf, W_co):
    from concourse.bass_utils import run_bass_kernel_spmd

    im2col, lhsT, peep = _host_prep(X, Wconv, bconv, W_ci, W_cf, W_co)
    nc = _get_nc()
    in_maps = [
        {"im2col": im2col[c], "lhsT": lhsT, "peep": peep[c]} for c in range(NC)
    ]
    trace = bool(os.environ.get("QRNN_TRACE"))
    res = run_bass_kernel_spmd(
        nc, in_maps, core_ids=list(range(NC)), trace=trace
    )
    LAST_RESULTS["exec_time_ns"] = getattr(res, "exec_time_ns", None)

    O = np.empty((B, COUT, T, H, W), np.float32)
    for c in range(NC):
        o = np.asarray(res.results[c]["out"], f16).astype(np.float32)
        o = o.reshape(T, 2, 64, 2, HS, W).transpose(1, 3, 2, 0, 4, 5)
        O[:, :, :, 8 * c:8 * c + HS, :] = o.reshape(B, COUT, T, HS, W)
    return O


# revision 18
# speedup vs baseline: 1.2236x; 1.0742x over previous
"""ConvQRNN Trainium2 kernel.

Strategy (8 NeuronCores, spatial H-sharding, 8 rows/core):
  - Conv3d(k=(2,3,3), CIN=3 -> 256) lowered to matmul: host builds a fp16
    im2col with K=56 rows (54 taps + ones row carrying the conv bias + one
    zero pad row).
  - Scan layout: [128, 1024] fp16 per step with partition = (b//2)*64 + ch,
    free = (b%2)*512 + h*64 + w.  The two column halves (q = b%2) are
    independent scans interleaved as two dependency chains.
  - C state lives at a FIXED address in ctb = [C0|tg0|C1|tg1]; tanh(g) is
    written next to C so the fused [s_f|s_i]*[C|tanh(g)] multiply reads one
    contiguous operand.  All fp16 DVE ops keep src0/src1/dst congruent
    mod 4KB, which HW requires for the 2x DVE mode.
  - o-gate: DVE writes Wco*C into the o PSUM bank, then the o-gate conv
    matmul ACCUMULATES on top (start=False), so sigmoid reads the finished
    pre-activation straight from PSUM.  No separate o-gate add.
  - Off-chain ops (tanh g, tanh C, sigmoid o, H-mul) are merged across the
    two halves into single 1024-col ops and deferred one step so they never
    stall the recurrence chain.
"""

import os

import numpy as np

B, CIN, T, H, W = 4, 3, 32, 64, 64
COUT = 64
NC = 8
HS = H // NC
K = 56
PIX = B * HS * W          # 2048
F = PIX // 2              # 1024
FH = F // 2               # 512
KAPPA = 8
NW = T // KAPPA

f16 = np.float16

_CACHE = {}
LAST_RESULTS = {}


def _host_prep(X, Wconv, bconv, W_ci, W_cf, W_co):
    X = np.ascontiguousarray(np.asarray(X, np.float32))
    Wconv = np.asarray(Wconv, np.float32)
    bconv = np.asarray(bconv, np.float32)
    Xp = np.pad(X, ((0, 0), (0, 0), (1, 0), (1, 1), (1, 1)))

    im2col = np.zeros((NC, K, T, PIX), f16)
    for c in range(NC):
        for cin in range(CIN):
            for dt in range(2):
                for dh in range(3):
                    for dw in range(3):
                        k = ((cin * 2 + dt) * 3 + dh) * 3 + dw
                        blk = Xp[:, cin, dt:dt + T,
                                 8 * c + dh:8 * c + dh + HS, dw:dw + W]
                        blk = blk.reshape(2, 2, T, HS, W).transpose(2, 0, 1, 3, 4)
                        im2col[c, k] = blk.reshape(T, PIX).astype(f16)
        im2col[c, 54] = 1.0

    # kernel gate order: (f, i, g, o)
    gate_order = (1, 0, 2, 3)
    lhsT = np.zeros((4, K, 128), f16)
    Wr = Wconv.reshape(4, COUT, CIN, 2, 3, 3)
    for gi, g in enumerate(gate_order):
        wk = Wr[g].transpose(1, 2, 3, 4, 0).reshape(54, COUT).astype(f16)
        lhsT[gi, :54, :64] = wk
        lhsT[gi, :54, 64:] = wk
        lhsT[gi, 54, :64] = bconv[g * 64:(g + 1) * 64].astype(f16)
        lhsT[gi, 54, 64:] = bconv[g * 64:(g + 1) * 64].astype(f16)

    # peep[c]: [128, 2048] = [Wcf | Wci | Wco | Wco], rows duplicated over
    # the two row-halves (both hold the same 64 channels).
    peep = np.zeros((NC, 128, 4 * FH), f16)
    for c in range(NC):
        for i, Wc in enumerate((W_cf, W_ci, W_co, W_co)):
            sl = np.asarray(Wc, np.float32)[:, 8 * c:8 * c + HS, :]
            sl = sl.reshape(64, FH).astype(f16)
            peep[c, :64, i * FH:(i + 1) * FH] = sl
            peep[c, 64:, i * FH:(i + 1) * FH] = sl
    return im2col, lhsT, peep


def _build_nc():
    import concourse.bacc as bacc
    import concourse.mybir as mybir
    from concourse.tile import TileContext

    fp16 = mybir.dt.float16
    fp32 = mybir.dt.float32
    AF = mybir.ActivationFunctionType

    nc = bacc.Bacc(None, target_bir_lowering=False)

    im2col_d = nc.dram_tensor("im2col", [K, T, PIX], fp16, kind="ExternalInput")
    lhsT_d = nc.dram_tensor("lhsT", [4, K, 128], fp16, kind="ExternalInput")
    peep_d = nc.dram_tensor("peep", [128, 4 * FH], fp16, kind="ExternalInput")
    out_d = nc.dram_tensor("out", [T, 128, F], fp16, kind="ExternalOutput")

    with TileContext(nc) as tc:
        with (
            tc.tile_pool(name="const", bufs=1) as constp,
            tc.tile_pool(name="al", bufs=1) as alp,
            tc.tile_pool(name="rhs", bufs=3) as rhsp,
            tc.tile_pool(name="psum", bufs=1, space="PSUM") as psump,
        ):
            wcif = constp.tile([128, F], fp16)       # [Wcf | Wci]
            wcoD = constp.tile([128, F], fp16)       # [Wco | Wco]
            nc.sync.dma_start(out=wcif[:], in_=peep_d[:, 0:F])
            nc.sync.dma_start(out=wcoD[:], in_=peep_d[:, F:2 * F])
            lhsT_sb = constp.tile([K, 4 * 128], fp16)
            nc.sync.dma_start(
                out=lhsT_sb[:].rearrange("k (g m) -> k g m", g=4),
                in_=lhsT_d[:].rearrange("g k m -> k g m"),
            )

            # aligned arena: every tile a 4KB multiple so all bases (and
            # equal-offset slices) stay congruent mod 4KB -> DVE 2x mode
            vv = alp.tile([128, 2 * F], fp16)        # [vf0|vi0|vf1|vi1]
            ss = alp.tile([128, 2 * F], fp16)        # sigmoid outputs
            ctb = alp.tile([128, 2 * F], fp16)       # [C0|tg0|C1|tg1]
            soh = alp.tile([128, KAPPA * F], fp16)   # sigmoid(a_o) slots
            tch = alp.tile([128, KAPPA * F], fp16)   # tanh(C) slots
            h8 = [alp.tile([128, KAPPA * F], fp16, name=f"h8{p}")
                  for p in range(2)]

            e_if = psump.tile([128, 2 * F], fp32)    # [f0|i0|f1|i1]
            e_g = psump.tile([128, F], fp32)         # [g0|g1]
            e_o = psump.tile([128, F], fp32)         # [o0|o1]

            nc.vector.memset(ctb[:, 0:FH], 0.0)
            nc.vector.memset(ctb[:, F:F + FH], 0.0)

            def cslice(q):
                return ctb[:, q * F:q * F + FH]

            def c2seg():
                # [C0 | C1] as a 2-segment strided AP
                return ctb[:].rearrange("p (s f) -> p s f", s=2)[:, :, 0:FH]

            def tg2seg():
                # [tg0 | tg1]
                return ctb[:].rearrange("p (s f) -> p s f", s=2)[:, :, FH:F]

            rhs_t = {}
            for t in range(T):
                j = t % KAPPA
                w = t // KAPPA
                par = w % 2

                rhs = rhsp.tile([K, PIX], fp16)
                nc.sync.dma_start(out=rhs[:], in_=im2col_d[:, t, :])
                rhs_t[t] = rhs

                # ---- previous step's o-gate + output tail ----
                if t > 0:
                    pj = (t - 1) % KAPPA
                    ppar = ((t - 1) // KAPPA) % 2
                    # Wco*C(t-1) into the o PSUM banks (before o-matmuls)
                    nc.vector.tensor_mul(
                        out=e_o[:].rearrange("p (s f) -> p s f", s=2),
                        in0=wcoD[:].rearrange("p (s f) -> p s f", s=2),
                        in1=c2seg(),
                    )
                    # o-gate conv accumulates on top
                    for hf in range(2):
                        lw = lhsT_sb[:, 3 * 128 + 64 * hf:3 * 128 + 64 * hf + 64]
                        for q in range(2):
                            b = 2 * hf + q
                            nc.tensor.matmul(
                                e_o[64 * hf:64 * hf + 64, q * FH:(q + 1) * FH],
                                lw,
                                rhs_t[t - 1][:, b * FH:(b + 1) * FH],
                                start=False,
                                stop=True,
                                tile_position=(0, 64 * hf),
                            )
                    del rhs_t[t - 1]
                    # tanh(C(t-1)) -> tch slot (before Cn(t) overwrites C)
                    nc.scalar.activation(
                        tch[:, pj * F:(pj + 1) * F],
                        c2seg(), AF.Tanh,
                    )
                    # sigmoid(a_o) straight from PSUM
                    nc.scalar.activation(
                        soh[:, pj * F:(pj + 1) * F], e_o[:], AF.Sigmoid
                    )
                    # H chunk (DVE; GpSimd activity halves DVE throughput)
                    nc.vector.tensor_mul(
                        out=h8[ppar][:, pj * F:(pj + 1) * F],
                        in0=soh[:, pj * F:(pj + 1) * F],
                        in1=tch[:, pj * F:(pj + 1) * F],
                    )
                    if pj == KAPPA - 1:
                        k0 = ((t - 1) // KAPPA) * KAPPA
                        for q in range(2):
                            nc.sync.dma_start(
                                out=out_d[k0:k0 + KAPPA, :, q * FH:(q + 1) * FH]
                                .rearrange("t p f -> p t f"),
                                in_=h8[ppar][:]
                                .rearrange("p (t s f) -> p t s f",
                                           t=KAPPA, s=2)[:, :, q, :],
                            )

                # ---- peephole terms straight into PSUM; conv accumulates ----
                for q in range(2):
                    # e_if[q] = [Wcf|Wci] * [C;C]  (f/i matmuls add on top)
                    nc.vector.tensor_mul(
                        out=e_if[:, q * F:(q + 1) * F]
                        .rearrange("p (o f) -> p o f", o=2),
                        in0=wcif[:].rearrange("p (o f) -> p o f", o=2),
                        in1=cslice(q).rearrange("p (o f) -> p o f", o=1)
                        .broadcast_to([128, 2, FH]),
                    )

                # ---- this step's g then f/i conv matmuls ----
                for gi in (2, 0, 1):
                    for hf in range(2):
                        lw = lhsT_sb[:, gi * 128 + 64 * hf:gi * 128 + 64 * hf + 64]
                        for q in range(2):
                            if gi < 2:
                                tgt = e_if[64 * hf:64 * hf + 64,
                                           q * F + gi * FH:q * F + (gi + 1) * FH]
                            else:
                                tgt = e_g[64 * hf:64 * hf + 64,
                                          q * FH:(q + 1) * FH]
                            b = 2 * hf + q
                            nc.tensor.matmul(
                                tgt, lw, rhs[:, b * FH:(b + 1) * FH],
                                start=(gi == 2), stop=True,
                                tile_position=(0, 64 * hf),
                            )

                # tanh(g) -> tg slots of ctb (2-segment dst)
                nc.scalar.activation(tg2seg(), e_g[:], AF.Tanh)

                # ---- recurrence chains (per half q) ----
                for q in range(2):
                    nc.scalar.activation(
                        ss[:, q * F:(q + 1) * F],
                        e_if[:, q * F:(q + 1) * F], AF.Sigmoid,
                    )
                for q in range(2):
                    # pp = [s_f|s_i] * [C|tg]
                    nc.vector.tensor_mul(
                        out=vv[:, q * F:(q + 1) * F],
                        in0=ss[:, q * F:(q + 1) * F],
                        in1=ctb[:, q * F:(q + 1) * F],
                    )
                    # C_next = s_f*C + s_i*tg
                    nc.vector.tensor_add(
                        out=cslice(q),
                        in0=vv[:, q * F:q * F + FH],
                        in1=vv[:, q * F + FH:(q + 1) * F],
                    )

            # ---- epilogue: last step's o-gate + tail ----
            t = T - 1
            pj = t % KAPPA
            ppar = (t // KAPPA) % 2
            nc.vector.tensor_mul(
                out=e_o[:].rearrange("p (s f) -> p s f", s=2),
                in0=wcoD[:].rearrange("p (s f) -> p s f", s=2),
                in1=c2seg(),
            )
            for hf in range(2):
                lw = lhsT_sb[:, 3 * 128 + 64 * hf:3 * 128 + 64 * hf + 64]
                for q in range(2):
                    b = 2 * hf + q
                    nc.tensor.matmul(
                        e_o[64 * hf:64 * hf + 64, q * FH:(q + 1) * FH],
                        lw,
                        rhs_t[t][:, b * FH:(b + 1) * FH],
                        start=False, stop=True,
                        tile_position=(0, 64 * hf),
                    )
            nc.scalar.activation(tch[:, pj * F:(pj + 1) * F], c2seg(), AF.Tanh)
            nc.scalar.activation(soh[:, pj * F:(pj + 1) * F], e_o[:], AF.Sigmoid)
            nc.vector.tensor_mul(
                out=h8[ppar][:, pj * F:(pj + 1) * F],
                in0=soh[:, pj * F:(pj + 1) * F],
                in1=tch[:, pj * F:(pj + 1) * F],
            )
            k0 = (NW - 1) * KAPPA
            for q in range(2):
                nc.sync.dma_start(
                    out=out_d[k0:k0 + KAPPA, :, q * FH:(q + 1) * FH]
                    .rearrange("t p f -> p t f"),
                    in_=h8[ppar][:]
                    .rearrange("p (t s f) -> p t s f", t=KAPPA, s=2)[:, :, q, :],
                )

    nc.compile()
    return nc


def _get_nc():
    if "nc" not in _CACHE:
        _CACHE["nc"] = _build_nc()
    return _CACHE["nc"]


def kernel(X, Wconv, bconv, W_ci, W_cf, W_co):
    from concourse.bass_utils import run_bass_kernel_spmd

    im2col, lhsT, peep = _host_prep(X, Wconv, bconv, W_ci, W_cf, W_co)
    nc = _get_nc()
    in_maps = [
        {"im2col": im2col[c], "lhsT": lhsT, "peep": peep[c]} for c in range(NC)
    ]
    trace = bool(os.environ.get("QRNN_TRACE"))
    res = run_bass_kernel_spmd(
        nc, in_maps, core_ids=list(range(NC)), trace=trace
    )
    LAST_RESULTS["exec_time_ns"] = getattr(res, "exec_time_ns", None)

    O = np.empty((B, COUT, T, H, W), np.float32)
    for c in range(NC):
        o = np.asarray(res.results[c]["out"], f16).astype(np.float32)
        o = o.reshape(T, 2, 64, 2, HS, W).transpose(1, 3, 2, 0, 4, 5)
        O[:, :, :, 8 * c:8 * c + HS, :] = o.reshape(B, COUT, T, HS, W)
    return O


# revision 19
# speedup vs baseline: 1.2622x; 1.0315x over previous
"""ConvQRNN Trainium2 kernel.

Strategy (8 NeuronCores, spatial H-sharding, 8 rows/core):
  - Conv3d(k=(2,3,3), CIN=3 -> 256) lowered to matmul: host builds a fp16
    im2col with K=56 rows (54 taps + ones row carrying the conv bias + one
    zero pad row).
  - Scan layout: [128, 1024] fp16 per step with partition = (b//2)*64 + ch,
    free = (b%2)*512 + h*64 + w.  The two column halves (q = b%2) are
    independent scans interleaved as two dependency chains.
  - C state lives at a FIXED address in ctb = [C0|tg0|C1|tg1]; tanh(g) is
    written next to C so the fused [s_f|s_i]*[C|tanh(g)] multiply reads one
    contiguous operand.  All fp16 DVE ops keep src0/src1/dst congruent
    mod 4KB, which HW requires for the 2x DVE mode.
  - o-gate: DVE writes Wco*C into the o PSUM bank, then the o-gate conv
    matmul ACCUMULATES on top (start=False), so sigmoid reads the finished
    pre-activation straight from PSUM.  No separate o-gate add.
  - Off-chain ops (tanh g, tanh C, sigmoid o, H-mul) are merged across the
    two halves into single 1024-col ops and deferred one step so they never
    stall the recurrence chain.
"""

import os

import numpy as np

B, CIN, T, H, W = 4, 3, 32, 64, 64
COUT = 64
NC = 8
HS = H // NC
K = 56
PIX = B * HS * W          # 2048
F = PIX // 2              # 1024
FH = F // 2               # 512
KAPPA = 8
NW = T // KAPPA

f16 = np.float16

_CACHE = {}
LAST_RESULTS = {}


def _host_prep(X, Wconv, bconv, W_ci, W_cf, W_co):
    X = np.ascontiguousarray(np.asarray(X, np.float32))
    Wconv = np.asarray(Wconv, np.float32)
    bconv = np.asarray(bconv, np.float32)
    Xp = np.pad(X, ((0, 0), (0, 0), (1, 0), (1, 1), (1, 1)))

    im2col = np.zeros((NC, K, T, PIX), f16)
    for c in range(NC):
        for cin in range(CIN):
            for dt in range(2):
                for dh in range(3):
                    for dw in range(3):
                        k = ((cin * 2 + dt) * 3 + dh) * 3 + dw
                        blk = Xp[:, cin, dt:dt + T,
                                 8 * c + dh:8 * c + dh + HS, dw:dw + W]
                        blk = blk.reshape(2, 2, T, HS, W).transpose(2, 0, 1, 3, 4)
                        im2col[c, k] = blk.reshape(T, PIX).astype(f16)
        im2col[c, 54] = 1.0

    # kernel gate order: (f, i, g, o)
    gate_order = (1, 0, 2, 3)
    lhsT = np.zeros((4, K, 128), f16)
    Wr = Wconv.reshape(4, COUT, CIN, 2, 3, 3)
    for gi, g in enumerate(gate_order):
        wk = Wr[g].transpose(1, 2, 3, 4, 0).reshape(54, COUT).astype(f16)
        lhsT[gi, :54, :64] = wk
        lhsT[gi, :54, 64:] = wk
        lhsT[gi, 54, :64] = bconv[g * 64:(g + 1) * 64].astype(f16)
        lhsT[gi, 54, 64:] = bconv[g * 64:(g + 1) * 64].astype(f16)

    # peep[c]: [128, 2048] = [Wcf | Wci | Wco | Wco], rows duplicated over
    # the two row-halves (both hold the same 64 channels).
    peep = np.zeros((NC, 128, 4 * FH), f16)
    for c in range(NC):
        for i, Wc in enumerate((W_cf, W_ci, W_co, W_co)):
            sl = np.asarray(Wc, np.float32)[:, 8 * c:8 * c + HS, :]
            sl = sl.reshape(64, FH).astype(f16)
            peep[c, :64, i * FH:(i + 1) * FH] = sl
            peep[c, 64:, i * FH:(i + 1) * FH] = sl
    return im2col, lhsT, peep


def _build_nc():
    import concourse.bacc as bacc
    import concourse.mybir as mybir
    from concourse.tile import TileContext

    fp16 = mybir.dt.float16
    fp32 = mybir.dt.float32
    AF = mybir.ActivationFunctionType

    nc = bacc.Bacc(None, target_bir_lowering=False)

    im2col_d = nc.dram_tensor("im2col", [K, T, PIX], fp16, kind="ExternalInput")
    lhsT_d = nc.dram_tensor("lhsT", [4, K, 128], fp16, kind="ExternalInput")
    peep_d = nc.dram_tensor("peep", [128, 4 * FH], fp16, kind="ExternalInput")
    out_d = nc.dram_tensor("out", [T, 128, F], fp16, kind="ExternalOutput")

    with TileContext(nc) as tc:
        with (
            tc.tile_pool(name="const", bufs=1) as constp,
            tc.tile_pool(name="al", bufs=1) as alp,
            tc.tile_pool(name="rhs", bufs=3) as rhsp,
            tc.tile_pool(name="psum", bufs=1, space="PSUM") as psump,
        ):
            wcif = constp.tile([128, F], fp16)       # [Wcf | Wci]
            wcoD = constp.tile([128, F], fp16)       # [Wco | Wco]
            nc.sync.dma_start(out=wcif[:], in_=peep_d[:, 0:F])
            nc.sync.dma_start(out=wcoD[:], in_=peep_d[:, F:2 * F])
            lhsT_sb = constp.tile([K, 4 * 128], fp16)
            nc.sync.dma_start(
                out=lhsT_sb[:].rearrange("k (g m) -> k g m", g=4),
                in_=lhsT_d[:].rearrange("g k m -> k g m"),
            )

            # aligned arena: every tile a 4KB multiple so all bases (and
            # equal-offset slices) stay congruent mod 4KB -> DVE 2x mode
            vv = alp.tile([128, 2 * F], fp16)        # [vf0|vi0|vf1|vi1]
            ss = alp.tile([128, 2 * F], fp16)        # sigmoid outputs
            ctb = alp.tile([128, 2 * F], fp16)       # [C0|tg0|C1|tg1]
            soh = alp.tile([128, KAPPA * F], fp16)   # sigmoid(a_o) slots
            tch = alp.tile([128, KAPPA * F], fp16)   # tanh(C) slots
            h8 = [alp.tile([128, KAPPA * F], fp16, name=f"h8{p}")
                  for p in range(2)]

            e_if = psump.tile([128, 2 * F], fp32)    # [f0|i0|f1|i1]
            e_g = psump.tile([128, F], fp32)         # [g0|g1]
            e_o = psump.tile([128, F], fp32)         # [o0|o1]

            nc.vector.memset(ctb[:, 0:FH], 0.0)
            nc.vector.memset(ctb[:, F:F + FH], 0.0)

            def cslice(q):
                return ctb[:, q * F:q * F + FH]

            def c2seg():
                # [C0 | C1] as a 2-segment strided AP
                return ctb[:].rearrange("p (s f) -> p s f", s=2)[:, :, 0:FH]

            def tg2seg():
                # [tg0 | tg1]
                return ctb[:].rearrange("p (s f) -> p s f", s=2)[:, :, FH:F]

            rhs_t = {}
            for t in range(T):
                j = t % KAPPA
                w = t // KAPPA
                par = w % 2

                rhs = rhsp.tile([K, PIX], fp16)
                nc.sync.dma_start(out=rhs[:], in_=im2col_d[:, t, :])
                rhs_t[t] = rhs

                pj = (t - 1) % KAPPA
                ppar = ((t - 1) // KAPPA) % 2

                # ---- DVE: peephole terms into PSUM (f/i conv adds on top),
                #      then Wco*C(t-1) into the o banks ----
                for q in range(2):
                    nc.vector.tensor_mul(
                        out=e_if[:, q * F:(q + 1) * F]
                        .rearrange("p (o f) -> p o f", o=2),
                        in0=wcif[:].rearrange("p (o f) -> p o f", o=2),
                        in1=cslice(q).rearrange("p (o f) -> p o f", o=1)
                        .broadcast_to([128, 2, FH]),
                    )
                if t > 0:
                    nc.vector.tensor_mul(
                        out=e_o[:].rearrange("p (s f) -> p s f", s=2),
                        in0=wcoD[:].rearrange("p (s f) -> p s f", s=2),
                        in1=c2seg(),
                    )

                # ---- PE: g first (dep-free), then f/i q0, f/i q1, then the
                #      previous step's o-gate accumulate ----
                def mm(gi, hf, q, which_rhs, start):
                    lw = lhsT_sb[:, gi * 128 + 64 * hf:gi * 128 + 64 * hf + 64]
                    if gi < 2:
                        tgt = e_if[64 * hf:64 * hf + 64,
                                   q * F + gi * FH:q * F + (gi + 1) * FH]
                    elif gi == 2:
                        tgt = e_g[64 * hf:64 * hf + 64, q * FH:(q + 1) * FH]
                    else:
                        tgt = e_o[64 * hf:64 * hf + 64, q * FH:(q + 1) * FH]
                    b = 2 * hf + q
                    nc.tensor.matmul(
                        tgt, lw, which_rhs[:, b * FH:(b + 1) * FH],
                        start=start, stop=True, tile_position=(0, 64 * hf),
                    )

                for hf in range(2):
                    for q in range(2):
                        mm(2, hf, q, rhs, True)
                for q in range(2):
                    for gi in (0, 1):
                        for hf in range(2):
                            mm(gi, hf, q, rhs, False)
                if t > 0:
                    for hf in range(2):
                        for q in range(2):
                            mm(3, hf, q, rhs_t[t - 1], False)
                    del rhs_t[t - 1]

                # ---- ACT: tc(t-1), tg(t), s0, s1, so(t-1) ----
                if t > 0:
                    nc.scalar.activation(
                        tch[:, pj * F:(pj + 1) * F], c2seg(), AF.Tanh,
                    )
                nc.scalar.activation(tg2seg(), e_g[:], AF.Tanh)
                for q in range(2):
                    nc.scalar.activation(
                        ss[:, q * F:(q + 1) * F],
                        e_if[:, q * F:(q + 1) * F], AF.Sigmoid,
                    )
                if t > 0:
                    nc.scalar.activation(
                        soh[:, pj * F:(pj + 1) * F], e_o[:], AF.Sigmoid
                    )

                # ---- DVE: pp/Cn chains, then H chunk of t-1 ----
                for q in range(2):
                    nc.vector.tensor_mul(
                        out=vv[:, q * F:(q + 1) * F],
                        in0=ss[:, q * F:(q + 1) * F],
                        in1=ctb[:, q * F:(q + 1) * F],
                    )
                    nc.vector.tensor_add(
                        out=cslice(q),
                        in0=vv[:, q * F:q * F + FH],
                        in1=vv[:, q * F + FH:(q + 1) * F],
                    )
                if t > 0:
                    nc.vector.tensor_mul(
                        out=h8[ppar][:, pj * F:(pj + 1) * F],
                        in0=soh[:, pj * F:(pj + 1) * F],
                        in1=tch[:, pj * F:(pj + 1) * F],
                    )
                    if pj == KAPPA - 1:
                        k0 = ((t - 1) // KAPPA) * KAPPA
                        for q in range(2):
                            nc.sync.dma_start(
                                out=out_d[k0:k0 + KAPPA, :, q * FH:(q + 1) * FH]
                                .rearrange("t p f -> p t f"),
                                in_=h8[ppar][:]
                                .rearrange("p (t s f) -> p t s f",
                                           t=KAPPA, s=2)[:, :, q, :],
                            )

            # ---- epilogue: last step's o-gate + tail ----
            t = T - 1
            pj = t % KAPPA
            ppar = (t // KAPPA) % 2
            nc.vector.tensor_mul(
                out=e_o[:].rearrange("p (s f) -> p s f", s=2),
                in0=wcoD[:].rearrange("p (s f) -> p s f", s=2),
                in1=c2seg(),
            )
            for hf in range(2):
                lw = lhsT_sb[:, 3 * 128 + 64 * hf:3 * 128 + 64 * hf + 64]
                for q in range(2):
                    b = 2 * hf + q
                    nc.tensor.matmul(
                        e_o[64 * hf:64 * hf + 64, q * FH:(q + 1) * FH],
                        lw,
                        rhs_t[t][:, b * FH:(b + 1) * FH],
                        start=False, stop=True,
                        tile_position=(0, 64 * hf),
                    )
            nc.scalar.activation(tch[:, pj * F:(pj + 1) * F], c2seg(), AF.Tanh)
            nc.scalar.activation(soh[:, pj * F:(pj + 1) * F], e_o[:], AF.Sigmoid)
            nc.vector.tensor_mul(
                out=h8[ppar][:, pj * F:(pj + 1) * F],
                in0=soh[:, pj * F:(pj + 1) * F],
                in1=tch[:, pj * F:(pj + 1) * F],
            )
            k0 = (NW - 1) * KAPPA
            for q in range(2):
                nc.sync.dma_start(
                    out=out_d[k0:k0 + KAPPA, :, q * FH:(q + 1) * FH]
                    .rearrange("t p f -> p t f"),
                    in_=h8[ppar][:]
                    .rearrange("p (t s f) -> p t s f", t=KAPPA, s=2)[:, :, q, :],
                )

    nc.compile()
    return nc


def _get_nc():
    if "nc" not in _CACHE:
        _CACHE["nc"] = _build_nc()
    return _CACHE["nc"]


def kernel(X, Wconv, bconv, W_ci, W_cf, W_co):
    from concourse.bass_utils import run_bass_kernel_spmd

    im2col, lhsT, peep = _host_prep(X, Wconv, bconv, W_ci, W_cf, W_co)
    nc = _get_nc()
    in_maps = [
        {"im2col": im2col[c], "lhsT": lhsT, "peep": peep[c]} for c in range(NC)
    ]
    trace = bool(os.environ.get("QRNN_TRACE"))
    res = run_bass_kernel_spmd(
        nc, in_maps, core_ids=list(range(NC)), trace=trace
    )
    LAST_RESULTS["exec_time_ns"] = getattr(res, "exec_time_ns", None)

    O = np.empty((B, COUT, T, H, W), np.float32)
    for c in range(NC):
        o = np.asarray(res.results[c]["out"], f16).astype(np.float32)
        o = o.reshape(T, 2, 64, 2, HS, W).transpose(1, 3, 2, 0, 4, 5)
        O[:, :, :, 8 * c:8 * c + HS, :] = o.reshape(B, COUT, T, HS, W)
    return O
